# revision 11
# baseline (speedup 1.0000x reference)
"""4D circular cross-correlation (qcd_ml C_Convolution, k=3, nd=4) on 8 TRN2 cores.

Math: out[o, x,y,z,t, s,c] = b[o] + sum_{i, ax,ay,az,at} W[i,o,ax,ay,az,at]
                                   * U[i, x+ax-1, y+ay-1, z+az-1, t+at-1, s,c]
(all site indices circular). U complex64 (4,16,16,16,32,4,3), W complex64
(4,4,3,3,3,3), b complex64 (4,).

Device mapping (per core, T sharded 8-way with +-1 halos prepared on host):
  - contraction (matmul partition) dim = (reim_in 2, C_in 4, X 16) = 128
  - output (PSUM partition) dim       = (reim_out 2, C_out 4, X0 16) = 128
  - X offsets (ax) live inside 27 stationary 128x128 matrices (one per
    (ay,az,at)), circularly banded in (x, x0); complex arithmetic is the
    2x2 [[Wr, Wi], [-Wi, Wr]] block over the reim axes.
  - moving free dim = (z_half 8, t_loc 4, spin*color 12) = 384 <= 512 (one
    PSUM bank); 27 offset matmuls accumulate into one PSUM bank per chunk.
  - y,z circular handled by host padding to 18; t halo from neighbor T-slab.
"""

import os
import sys
import itertools
import numpy as np

for _p in ("/opt/trn_rl_repo",):
    if _p not in sys.path and os.path.isdir(_p):
        sys.path.insert(0, _p)

C_IN, C_OUT = 4, 4
X = Y = Z = 16
T = 32
SC = 12  # spin*color
NCORES = 8
TLOC = T // NCORES          # 4
TPAD = TLOC + 2             # 6
YPAD, ZPAD = Y + 2, Z + 2   # 18
UH_ROWS = 10                # y_pad rows per half tile (0..9 / 8..17)
OFFSETS = list(itertools.product(range(3), repeat=3))  # (ay, az, at)
NOFF = len(OFFSETS)         # 27
FREE = 8 * TLOC * SC        # 384, one chunk = (z_half, t, sc)

USE_FP32R = os.environ.get("CONV_FP32R", "1") == "1"


def _prep_u_shards(U):
    """U complex (4,16,16,16,32,4,3) -> per-core float32 [128, YPAD,ZPAD,TPAD,SC]
    split into two y-halves [128, 10, ZPAD, TPAD, SC]."""
    Ur = np.stack([U.real, U.imag], axis=0).astype(np.float32)  # (2,4,X,Y,Z,T,4,3)
    Ur = Ur.reshape(2, C_IN, X, Y, Z, T, SC)
    Up = np.pad(Ur, ((0, 0), (0, 0), (0, 0), (1, 1), (1, 1), (0, 0), (0, 0)),
                mode="wrap")  # (2,4,16,18,18,32,12)
    shards = []
    for k in range(NCORES):
        t0 = k * TLOC
        tidx = np.arange(t0 - 1, t0 + TLOC + 1) % T
        s = np.take(Up, tidx, axis=5)               # (2,4,16,18,18,6,12)
        s = s.reshape(128, YPAD, ZPAD, TPAD, SC)
        u0 = np.ascontiguousarray(s[:, 0:UH_ROWS])
        u1 = np.ascontiguousarray(s[:, YPAD - UH_ROWS:YPAD])
        shards.append((u0, u1))
    return shards


def _prep_wstat(W):
    """W complex (4,4,3,3,3,3) -> [128, 27, 128] float32 stationary stack.

    wstat[(riI,i,x), a, (riO,o,x0)] = M[riI,riO] W_{.}[i,o,ax,ay,az,at]
    with ax = (x - x0 + 1) mod 16 when in {0,1,2}, else 0;
    M = [[Wr, Wi], [-Wi, Wr]] (columns riO: out_r, out_i).
    """
    Wr = np.ascontiguousarray(W.real).astype(np.float32)
    Wi = np.ascontiguousarray(W.imag).astype(np.float32)
    stat = np.zeros((2, C_IN, X, NOFF, 2, C_OUT, X), np.float32)
    for aidx, (ay, az, at) in enumerate(OFFSETS):
        for ax in range(3):
            wr = Wr[:, :, ax, ay, az, at]  # (i, o)
            wi = Wi[:, :, ax, ay, az, at]
            for x0 in range(X):
                x = (x0 + ax - 1) % X
                stat[0, :, x, aidx, 0, :, x0] = wr
                stat[1, :, x, aidx, 0, :, x0] = -wi
                stat[0, :, x, aidx, 1, :, x0] = wi
                stat[1, :, x, aidx, 1, :, x0] = wr
    return np.ascontiguousarray(stat.reshape(128, NOFF, 128))


def _assemble(results, b):
    """results[k]["out"]: [128, Y, Z, TLOC, SC] f32 -> complex (4,16,16,16,32,4,3)."""
    out = np.empty((C_OUT, X, Y, Z, T, SC), np.complex64)
    for k in range(NCORES):
        r = np.asarray(results[k]["out"], np.float32).reshape(2, C_OUT, X, Y, Z, TLOC, SC)
        out[:, :, :, :, k * TLOC:(k + 1) * TLOC, :] = r[0] + 1j * r[1]
    out += np.asarray(b, np.complex64).reshape(C_OUT, 1, 1, 1, 1, 1)
    return np.ascontiguousarray(out.reshape(C_OUT, X, Y, Z, T, 4, 3))


def _build_nc():
    import concourse.mybir as mybir
    from concourse import bacc, tile
    from contextlib import ExitStack

    f32 = mybir.dt.float32
    mm_dt = mybir.dt.float32r if USE_FP32R else f32

    WCOLS = NOFF * 128          # 3456
    UCOLS = UH_ROWS * ZPAD * TPAD * SC  # 12960

    nc = bacc.Bacc()
    # Single fused input parameter: [wstat | u_half0 | u_half1] along the free
    # dim, loaded by ONE dma_start. The LDWEIGHTS half of a self-loading
    # fp32/fp32r matmul has a single sync-wait slot, so every matmul must
    # carry at most one semaphore wait; one input DMA (one HWDGE queue sem)
    # guarantees that for the input dependency.
    a_dram = nc.declare_dram_parameter("allin", [128, WCOLS + 2 * UCOLS], mm_dt, isOutput=False)
    o_dram = nc.declare_dram_parameter("out", [128, Y, Z, TLOC, SC], f32, isOutput=True)

    with tile.TileContext(nc) as tc, ExitStack() as ctx:
        ipool = ctx.enter_context(tc.tile_pool(name="inp", bufs=1))
        # bufs=4: one output tile slot per y-group, no slot reuse -> the
        # PSUM->SBUF copies never wait on an out-DMA slot release (engine
        # instructions have tiny HW sync-wait budgets: matmul 1, copy 2).
        opool = ctx.enter_context(tc.tile_pool(name="o", bufs=4))
        ppool = ctx.enter_context(tc.tile_pool(name="psum", bufs=6, space="PSUM"))

        big = ipool.tile([128, WCOLS + 2 * UCOLS], mm_dt)
        nc.sync.dma_start(big[:], a_dram[:])

        wt = big[:, :WCOLS].rearrange("p (a m) -> p a m", a=NOFF)
        ut = [
            big[:, WCOLS + h * UCOLS: WCOLS + (h + 1) * UCOLS].rearrange(
                "p (y z t s) -> p y z t s", y=UH_ROWS, z=ZPAD, t=TPAD)
            for h in range(2)
        ]

        # 4 output groups of 4 y-slabs each: 1 input + 4 output DMAs total,
        # so no HWDGE queue is reused (queue reuse adds a second sync wait on
        # the DMA trigger, which has a single wait slot).
        YG = 4
        for g in range(Y // YG):
            ot = opool.tile([128, YG, Z, TLOC, SC], f32)
            for yl_g in range(YG):
                y = g * YG + yl_g
                h = 0 if y < 8 else 1
                yl = y - 8 * h
                for zh in range(2):
                    pt = ppool.tile([128, FREE], f32)
                    for aidx, (ay, az, at) in enumerate(OFFSETS):
                        rhs = ut[h][:, yl + ay, az + zh * 8: az + zh * 8 + 8,
                                    at: at + TLOC, :]
                        nc.tensor.matmul(
                            pt[:],
                            wt[:, aidx, :],
                            rhs,
                            start=(aidx == 0),
                            stop=(aidx == NOFF - 1),
                        )
                    nc.vector.tensor_copy(ot[:, yl_g, zh * 8:(zh + 1) * 8], pt[:])
            nc.sync.dma_start(o_dram[:, g * YG:(g + 1) * YG], ot[:])

    # Bacc defers register allocation and sync-wait splitting to finalize();
    # run_bass_via_pjrt serializes the module as-is, so finalize here.
    nc.finalize()
    return nc


_NC_CACHE = None
LAST_RUN = None  # BassKernelResults of the most recent device run (for test.py)


def kernel(U, W, b):
    global _NC_CACHE, LAST_RUN
    shards = _prep_u_shards(np.asarray(U))
    wstat = _prep_wstat(np.asarray(W))

    if os.environ.get("CONV_EMULATE", "0") == "1":
        results = _emulate(shards, wstat)
    else:
        from concourse.bass_utils import run_bass_kernel_spmd
        if _NC_CACHE is None:
            _NC_CACHE = _build_nc()
        wflat = wstat.reshape(128, -1)
        in_maps = [
            {"allin": np.ascontiguousarray(np.concatenate(
                [wflat, u0.reshape(128, -1), u1.reshape(128, -1)], axis=1))}
            for (u0, u1) in shards
        ]
        trace = os.environ.get("CONV_TRACE", "0") == "1"
        LAST_RUN = run_bass_kernel_spmd(
            _NC_CACHE, in_maps, core_ids=list(range(NCORES)), trace=trace)
        results = LAST_RUN.results
    return _assemble(results, np.asarray(b))


def _emulate(shards, wstat):
    """Host-side bit-faithful emulation of the device matmul program."""
    results = []
    for (u0, u1) in shards:
        full = np.concatenate([u0, u1[:, 2:]], axis=1)  # [128, 18, ZPAD, TPAD, SC]
        acc = np.zeros((128, Y * Z * TLOC * SC), np.float32)
        for aidx, (ay, az, at) in enumerate(OFFSETS):
            slab = full[:, ay:ay + Y, az:az + Z, at:at + TLOC, :].reshape(128, -1)
            acc += wstat[:, aidx, :].T.astype(np.float64) @ slab.astype(np.float64)
        results.append({"out": acc.reshape(128, Y, Z, TLOC, SC)})
    return results


# revision 14
# speedup vs baseline: 1.1071x; 1.1071x over previous
"""4D circular cross-correlation (qcd_ml C_Convolution, k=3, nd=4) on 8 TRN2 cores.

Math: out[o, x,y,z,t, s,c] = b[o] + sum_{i, ax,ay,az,at} W[i,o,ax,ay,az,at]
                                   * U[i, x+ax-1, y+ay-1, z+az-1, t+at-1, s,c]
(all site indices circular). U complex64 (4,16,16,16,32,4,3), W complex64
(4,4,3,3,3,3), b complex64 (4,).

Device mapping (per core, T sharded 8-way with +-1 halos prepared on host):
  - contraction (matmul partition) dim = (reim_in 2, C_in 4, X 16) = 128
  - output (PSUM partition) dim       = (reim_out 2, C_out 4, X0 16) = 128
  - X offsets (ax) live inside 27 stationary 128x128 matrices (one per
    (ay,az,at)), circularly banded in (x, x0); complex arithmetic is the
    2x2 [[Wr, Wi], [-Wi, Wr]] block over the reim axes.
  - moving free dim = (z_half 8, t_loc 4, spin*color 12) = 384 <= 512 (one
    PSUM bank); 27 offset matmuls accumulate into one PSUM bank per chunk.
  - y,z circular handled by host padding to 18; t halo from neighbor T-slab.
"""

import os
import sys
import itertools
import numpy as np

for _p in ("/opt/trn_rl_repo",):
    if _p not in sys.path and os.path.isdir(_p):
        sys.path.insert(0, _p)

C_IN, C_OUT = 4, 4
X = Y = Z = 16
T = 32
SC = 12  # spin*color
NCORES = 8
TLOC = T // NCORES          # 4
TPAD = TLOC + 2             # 6
YPAD, ZPAD = Y + 2, Z + 2   # 18
UH_ROWS = 10                # y_pad rows per half tile (0..9 / 8..17)
OFFSETS = list(itertools.product(range(3), repeat=3))  # (ay, az, at)
NOFF = len(OFFSETS)         # 27
FREE = 8 * TLOC * SC        # 384, one chunk = (z_half, t, sc)

USE_FP32R = os.environ.get("CONV_FP32R", "1") == "1"


def _prep_u_shards(U):
    """U complex (4,16,16,16,32,4,3) -> per-core float32 [128, YPAD,ZPAD,TPAD,SC]
    split into two y-halves [128, 10, ZPAD, TPAD, SC]."""
    Ur = np.stack([U.real, U.imag], axis=0).astype(np.float32)  # (2,4,X,Y,Z,T,4,3)
    Ur = Ur.reshape(2, C_IN, X, Y, Z, T, SC)
    Up = np.pad(Ur, ((0, 0), (0, 0), (0, 0), (1, 1), (1, 1), (0, 0), (0, 0)),
                mode="wrap")  # (2,4,16,18,18,32,12)
    shards = []
    for k in range(NCORES):
        t0 = k * TLOC
        tidx = np.arange(t0 - 1, t0 + TLOC + 1) % T
        s = np.take(Up, tidx, axis=5)               # (2,4,16,18,18,6,12)
        s = s.reshape(128, YPAD, ZPAD, TPAD, SC)
        u0 = np.ascontiguousarray(s[:, 0:UH_ROWS])
        u1 = np.ascontiguousarray(s[:, YPAD - UH_ROWS:YPAD])
        shards.append((u0, u1))
    return shards


def _prep_wstat(W):
    """W complex (4,4,3,3,3,3) -> [128, 27, 128] float32 stationary stack.

    wstat[(riI,i,x), a, (riO,o,x0)] = M[riI,riO] W_{.}[i,o,ax,ay,az,at]
    with ax = (x - x0 + 1) mod 16 when in {0,1,2}, else 0;
    M = [[Wr, Wi], [-Wi, Wr]] (columns riO: out_r, out_i).
    """
    Wr = np.ascontiguousarray(W.real).astype(np.float32)
    Wi = np.ascontiguousarray(W.imag).astype(np.float32)
    stat = np.zeros((2, C_IN, X, NOFF, 2, C_OUT, X), np.float32)
    for aidx, (ay, az, at) in enumerate(OFFSETS):
        for ax in range(3):
            wr = Wr[:, :, ax, ay, az, at]  # (i, o)
            wi = Wi[:, :, ax, ay, az, at]
            for x0 in range(X):
                x = (x0 + ax - 1) % X
                stat[0, :, x, aidx, 0, :, x0] = wr
                stat[1, :, x, aidx, 0, :, x0] = -wi
                stat[0, :, x, aidx, 1, :, x0] = wi
                stat[1, :, x, aidx, 1, :, x0] = wr
    return np.ascontiguousarray(stat.reshape(128, NOFF, 128))


def _assemble(results, b):
    """results[k]["out"]: [128, Y, Z, TLOC, SC] f32 -> complex (4,16,16,16,32,4,3)."""
    out = np.empty((C_OUT, X, Y, Z, T, SC), np.complex64)
    for k in range(NCORES):
        r = np.asarray(results[k]["out"], np.float32).reshape(2, C_OUT, X, Y, Z, TLOC, SC)
        out[:, :, :, :, k * TLOC:(k + 1) * TLOC, :] = r[0] + 1j * r[1]
    out += np.asarray(b, np.complex64).reshape(C_OUT, 1, 1, 1, 1, 1)
    return np.ascontiguousarray(out.reshape(C_OUT, X, Y, Z, T, 4, 3))


def _build_nc():
    import concourse.mybir as mybir
    from concourse import bacc, tile
    from contextlib import ExitStack

    f32 = mybir.dt.float32
    mm_dt = mybir.dt.float32r if USE_FP32R else f32

    WCOLS = NOFF * 128          # 3456
    UCOLS = UH_ROWS * ZPAD * TPAD * SC  # 12960

    nc = bacc.Bacc()
    # wstat+u_half0 fused in one DMA (compute on y 0..7 starts as soon as it
    # lands); u_half1 is a separate DMA that overlaps with first-half compute.
    # Bacc's generate_event_semaphores splits any multi-wait instruction, so
    # separate DMA queues are fine.
    a_dram = nc.declare_dram_parameter("wu0", [128, WCOLS + UCOLS], mm_dt, isOutput=False)
    u1_dram = nc.declare_dram_parameter("u1", [128, UH_ROWS, ZPAD, TPAD, SC], mm_dt, isOutput=False)
    o_dram = nc.declare_dram_parameter("out", [128, Y, Z, TLOC, SC], f32, isOutput=True)

    with tile.TileContext(nc) as tc, ExitStack() as ctx:
        ipool = ctx.enter_context(tc.tile_pool(name="inp", bufs=1))
        # bufs=4: one output tile slot per y-group, no slot reuse -> the
        # PSUM->SBUF copies never wait on an out-DMA slot release.
        opool = ctx.enter_context(tc.tile_pool(name="o", bufs=4))
        ppool = ctx.enter_context(tc.tile_pool(name="psum", bufs=6, space="PSUM"))

        big = ipool.tile([128, WCOLS + UCOLS], mm_dt, tag="wu0")
        nc.sync.dma_start(big[:], a_dram[:])
        u1t = ipool.tile([128, UH_ROWS, ZPAD, TPAD, SC], mm_dt, tag="u1")
        nc.sync.dma_start(u1t[:], u1_dram[:])

        wt = big[:, :WCOLS].rearrange("p (a m) -> p a m", a=NOFF)
        ut = [
            big[:, WCOLS:].rearrange(
                "p (y z t s) -> p y z t s", y=UH_ROWS, z=ZPAD, t=TPAD),
            u1t,
        ]

        # 4 output groups of 4 y-slabs each: 1 input + 4 output DMAs total,
        # so no HWDGE queue is reused (queue reuse adds a second sync wait on
        # the DMA trigger, which has a single wait slot).
        YG = 4
        for g in range(Y // YG):
            ot = opool.tile([128, YG, Z, TLOC, SC], f32)
            for yl_g in range(YG):
                y = g * YG + yl_g
                h = 0 if y < 8 else 1
                yl = y - 8 * h
                for zh in range(2):
                    pt = ppool.tile([128, FREE], f32)
                    for aidx, (ay, az, at) in enumerate(OFFSETS):
                        rhs = ut[h][:, yl + ay, az + zh * 8: az + zh * 8 + 8,
                                    at: at + TLOC, :]
                        nc.tensor.matmul(
                            pt[:],
                            wt[:, aidx, :],
                            rhs,
                            start=(aidx == 0),
                            stop=(aidx == NOFF - 1),
                        )
                    nc.vector.tensor_copy(ot[:, yl_g, zh * 8:(zh + 1) * 8], pt[:])
            nc.sync.dma_start(o_dram[:, g * YG:(g + 1) * YG], ot[:])

    # Bacc defers register allocation and sync-wait splitting to finalize();
    # run_bass_via_pjrt serializes the module as-is, so finalize here.
    nc.finalize()
    return nc


_NC_CACHE = None
LAST_RUN = None  # BassKernelResults of the most recent device run (for test.py)


def kernel(U, W, b):
    global _NC_CACHE, LAST_RUN
    shards = _prep_u_shards(np.asarray(U))
    wstat = _prep_wstat(np.asarray(W))

    if os.environ.get("CONV_EMULATE", "0") == "1":
        results = _emulate(shards, wstat)
    else:
        from concourse.bass_utils import run_bass_kernel_spmd
        if _NC_CACHE is None:
            _NC_CACHE = _build_nc()
        wflat = wstat.reshape(128, -1)
        in_maps = [
            {"wu0": np.ascontiguousarray(np.concatenate(
                [wflat, u0.reshape(128, -1)], axis=1)),
             "u1": u1}
            for (u0, u1) in shards
        ]
        trace = os.environ.get("CONV_TRACE", "0") == "1"
        LAST_RUN = run_bass_kernel_spmd(
            _NC_CACHE, in_maps, core_ids=list(range(NCORES)), trace=trace)
        results = LAST_RUN.results
    return _assemble(results, np.asarray(b))


def _emulate(shards, wstat):
    """Host-side bit-faithful emulation of the device matmul program."""
    results = []
    for (u0, u1) in shards:
        full = np.concatenate([u0, u1[:, 2:]], axis=1)  # [128, 18, ZPAD, TPAD, SC]
        acc = np.zeros((128, Y * Z * TLOC * SC), np.float32)
        for aidx, (ay, az, at) in enumerate(OFFSETS):
            slab = full[:, ay:ay + Y, az:az + Z, at:at + TLOC, :].reshape(128, -1)
            acc += wstat[:, aidx, :].T.astype(np.float64) @ slab.astype(np.float64)
        results.append({"out": acc.reshape(128, Y, Z, TLOC, SC)})
    return results


# revision 17
# speedup vs baseline: 1.3859x; 1.2519x over previous
"""4D circular cross-correlation (qcd_ml C_Convolution, k=3, nd=4) on 8 TRN2 cores.

Math: out[o, x,y,z,t, s,c] = b[o] + sum_{i, ax,ay,az,at} W[i,o,ax,ay,az,at]
                                   * U[i, x+ax-1, y+ay-1, z+az-1, t+at-1, s,c]
(all site indices circular). U complex64 (4,16,16,16,32,4,3), W complex64
(4,4,3,3,3,3), b complex64 (4,).

Device mapping (per core, T sharded 8-way with +-1 halos prepared on host):
  - contraction (matmul partition) dim = (reim_in 2, C_in 4, X 16) = 128
  - output (PSUM partition) dim       = (reim_out 2, C_out 4, X0 16) = 128
  - X offsets (ax) live inside the stationary 128x128 matrices, circularly
    banded in (x, x0); complex arithmetic is the 2x2 [[Wr, Wi], [-Wi, Wr]]
    block over the reim axes.
  - The T offsets (at) are removed by a host-side Winograd F(2,3) transform
    along t: U is transformed into 4 phases per 2-output t-tile
    (B^T d with d = U[t-1..t+2]); weights become G W (4 phases); the device
    accumulates 9 (ay,az) offsets per phase into PSUM and combines the 4
    phase results with A^T (4 DVE tensor ops per y-slab).
  - moving free dim = (z 16, t-tile 2, spin*color 12) = 384 <= 512 (one
    PSUM bank per phase).
  - y,z circular handled by host padding to 18; t halo from neighbor T-slab.
"""

import os
import sys
import itertools
import numpy as np

for _p in ("/opt/trn_rl_repo",):
    if _p not in sys.path and os.path.isdir(_p):
        sys.path.insert(0, _p)

C_IN, C_OUT = 4, 4
X = Y = Z = 16
T = 32
SC = 12  # spin*color
NCORES = 8
TLOC = T // NCORES          # 4
NTT = TLOC // 2             # 2 t-tiles of 2 outputs each
NPH = 4                     # Winograd F(2,3) phases
YPAD, ZPAD = Y + 2, Z + 2   # 18
UH_ROWS = 10                # y_pad rows per half tile (0..9 / 8..17)
OFF9 = list(itertools.product(range(3), repeat=2))  # (ay, az)
FREE = Z * NTT * SC         # 384, one chunk = (z, tt, sc) per phase

USE_FP32R = os.environ.get("CONV_FP32R", "1") == "1"

# Winograd F(2,3) matrices (correlation form: out[r] = sum_k g[k] d[r+k]).
BT = np.array([[1, 0, -1, 0],
               [0, 1, 1, 0],
               [0, -1, 1, 0],
               [0, 1, 0, -1]], np.float32)
G = np.array([[1, 0, 0],
              [0.5, 0.5, 0.5],
              [0.5, -0.5, 0.5],
              [0, 0, 1]], np.float32)
# A^T = [[1,1,1,0],[0,1,-1,-1]] applied on the device via DVE adds/subs.


def _prep_u_shards(U):
    """U complex (4,16,16,16,32,4,3) -> per-core float32 y-halves
    [128, UH_ROWS, ZPAD, NTT, NPH, SC] of the t-Winograd-transformed field."""
    Ur = np.stack([U.real, U.imag], axis=0).astype(np.float32)  # (2,4,X,Y,Z,T,4,3)
    Ur = Ur.reshape(2, C_IN, X, Y, Z, T, SC)
    Up = np.pad(Ur, ((0, 0), (0, 0), (0, 0), (1, 1), (1, 1), (0, 0), (0, 0)),
                mode="wrap")  # (2,4,16,18,18,32,12)
    shards = []
    for k in range(NCORES):
        t0 = k * TLOC
        # windows: tile tt covers outputs t0+2tt+{0,1}, needs t0+2tt-1..+2
        d = np.empty((2, C_IN, X, YPAD, ZPAD, NTT, 4, SC), np.float32)
        for tt in range(NTT):
            tidx = np.arange(t0 + 2 * tt - 1, t0 + 2 * tt + 3) % T
            d[:, :, :, :, :, tt] = np.moveaxis(
                np.take(Up, tidx, axis=5), 5, 5)  # (2,4,16,18,18,4,12)
        # phases: m = BT @ d  over the window axis
        m = np.einsum("pk,rixyztks->rixyztps", BT,
                      d.reshape(2, C_IN, X, YPAD, ZPAD, NTT, 4, SC))
        # m: (2,4,16,18,18,NTT,NPH,12) -> [128, y, z, tt, ph, sc]
        m = m.reshape(128, YPAD, ZPAD, NTT, NPH, SC)
        u0 = np.ascontiguousarray(m[:, 0:UH_ROWS])
        u1 = np.ascontiguousarray(m[:, YPAD - UH_ROWS:YPAD])
        shards.append((u0, u1))
    return shards


def _prep_wstat(W):
    """W complex (4,4,3,3,3,3) -> [128, NPH*9, 128] float32 stationary stack.

    For phase p and (ay,az): Wg[p][i,o,ax,ay,az] = sum_at G[p,at] W[..,at];
    band in (x,x0): ax = (x - x0 + 1) mod 16 in {0,1,2};
    ri block M = [[Wr, Wi], [-Wi, Wr]] (columns riO: out_r, out_i).
    """
    Wc = np.ascontiguousarray(W).astype(np.complex64)
    Wg = np.einsum("pk,ioxyzk->pioxyz", G.astype(np.complex64), Wc)  # (4,4,4,3,3,3)
    stat = np.zeros((2, C_IN, X, NPH * 9, 2, C_OUT, X), np.float32)
    for ph in range(NPH):
        for aidx, (ay, az) in enumerate(OFF9):
            widx = ph * 9 + aidx
            for ax in range(3):
                wr = Wg[ph, :, :, ax, ay, az].real
                wi = Wg[ph, :, :, ax, ay, az].imag
                for x0 in range(X):
                    x = (x0 + ax - 1) % X
                    stat[0, :, x, widx, 0, :, x0] = wr
                    stat[1, :, x, widx, 0, :, x0] = -wi
                    stat[0, :, x, widx, 1, :, x0] = wi
                    stat[1, :, x, widx, 1, :, x0] = wr
    return np.ascontiguousarray(stat.reshape(128, NPH * 9, 128))


def _assemble(results, b):
    """results[k]["out"]: [128, Y, Z, TLOC, SC] f32 -> complex (4,16,16,16,32,4,3)."""
    out = np.empty((C_OUT, X, Y, Z, T, SC), np.complex64)
    for k in range(NCORES):
        r = np.asarray(results[k]["out"], np.float32).reshape(2, C_OUT, X, Y, Z, TLOC, SC)
        out[:, :, :, :, k * TLOC:(k + 1) * TLOC, :] = r[0] + 1j * r[1]
    out += np.asarray(b, np.complex64).reshape(C_OUT, 1, 1, 1, 1, 1)
    return np.ascontiguousarray(out.reshape(C_OUT, X, Y, Z, T, 4, 3))


def _build_nc():
    import concourse.mybir as mybir
    from concourse import bacc, tile
    from contextlib import ExitStack

    f32 = mybir.dt.float32
    mm_dt = mybir.dt.float32r if USE_FP32R else f32

    WCOLS = NPH * 9 * 128                    # 4608
    UCOLS = UH_ROWS * ZPAD * NTT * NPH * SC  # 17280

    nc = bacc.Bacc()
    # wstat+u_half0 fused in one DMA (compute on y 0..7 starts as soon as it
    # lands); u_half1 is a separate DMA that overlaps with first-half compute
    # (HWDGE rings are FIFO, so the later-triggered u1 doesn't slow wu0).
    a_dram = nc.declare_dram_parameter("wu0", [128, WCOLS + UCOLS], mm_dt, isOutput=False)
    u1_dram = nc.declare_dram_parameter(
        "u1", [128, UH_ROWS, ZPAD, NTT, NPH, SC], mm_dt, isOutput=False)
    o_dram = nc.declare_dram_parameter("out", [128, Y, Z, TLOC, SC], f32, isOutput=True)

    with tile.TileContext(nc) as tc, ExitStack() as ctx:
        ipool = ctx.enter_context(tc.tile_pool(name="inp", bufs=1))
        opool = ctx.enter_context(tc.tile_pool(name="o", bufs=2))
        tpool = ctx.enter_context(tc.tile_pool(name="tmp", bufs=2))
        ppool = ctx.enter_context(tc.tile_pool(name="psum", bufs=8, space="PSUM"))

        big = ipool.tile([128, WCOLS + UCOLS], mm_dt, tag="wu0")
        nc.sync.dma_start(big[:], a_dram[:])
        u1t = ipool.tile([128, UH_ROWS, ZPAD, NTT, NPH, SC], mm_dt, tag="u1")
        nc.sync.dma_start(u1t[:], u1_dram[:])

        wt = big[:, :WCOLS].rearrange("p (a m) -> p a m", a=NPH * 9)
        ut = [
            big[:, WCOLS:].rearrange(
                "p (y z tt ph s) -> p y z tt ph s",
                y=UH_ROWS, z=ZPAD, tt=NTT, ph=NPH),
            u1t,
        ]

        YG = 4
        for g in range(Y // YG):
            # ot layout [z, tt, r, sc]: flattened (tt,r) == t_loc = 2*tt+r.
            ot = opool.tile([128, YG, Z, NTT, 2, SC], f32)
            for yl_g in range(YG):
                y = g * YG + yl_g
                h = 0 if y < 8 else 1
                yl = y - 8 * h
                pts = []
                for ph in range(NPH):
                    pt = ppool.tile([128, FREE], f32)
                    for aidx, (ay, az) in enumerate(OFF9):
                        rhs = ut[h][:, yl + ay, az: az + Z, :, ph, :]
                        nc.tensor.matmul(
                            pt[:],
                            wt[:, ph * 9 + aidx, :],
                            rhs,
                            start=(aidx == 0),
                            stop=(aidx == 8),
                        )
                    pts.append(pt)
                # A^T: out[2tt+0] = m0+m1+m2 ; out[2tt+1] = m1-m2-m3.
                # DVE ops may read at most ONE PSUM operand -> stage m1.
                m1s = tpool.tile([128, FREE], f32, tag="m1s")
                nc.vector.tensor_copy(m1s[:], pts[1][:])
                s0 = tpool.tile([128, FREE], f32, tag="s0")
                nc.vector.tensor_add(s0[:], m1s[:], pts[0][:])
                nc.vector.tensor_add(ot[:, yl_g, :, :, 0, :], s0[:], pts[2][:])
                s1 = tpool.tile([128, FREE], f32, tag="s1")
                nc.vector.tensor_sub(s1[:], m1s[:], pts[2][:])
                nc.vector.tensor_sub(ot[:, yl_g, :, :, 1, :], s1[:], pts[3][:])
            nc.sync.dma_start(o_dram[:, g * YG:(g + 1) * YG],
                              ot.rearrange("p yg z tt r s -> p yg z (tt r) s"))

    # Bacc defers register allocation and sync-wait splitting to finalize();
    # run_bass_via_pjrt serializes the module as-is, so finalize here.
    nc.finalize()
    return nc


_NC_CACHE = None
LAST_RUN = None  # BassKernelResults of the most recent device run (for test.py)


def kernel(U, W, b):
    global _NC_CACHE, LAST_RUN
    shards = _prep_u_shards(np.asarray(U))
    wstat = _prep_wstat(np.asarray(W))

    if os.environ.get("CONV_EMULATE", "0") == "1":
        results = _emulate(shards, wstat)
    else:
        from concourse.bass_utils import run_bass_kernel_spmd
        if _NC_CACHE is None:
            _NC_CACHE = _build_nc()
        wflat = wstat.reshape(128, -1)
        in_maps = [
            {"wu0": np.ascontiguousarray(np.concatenate(
                [wflat, u0.reshape(128, -1)], axis=1)),
             "u1": u1}
            for (u0, u1) in shards
        ]
        trace = os.environ.get("CONV_TRACE", "0") == "1"
        LAST_RUN = run_bass_kernel_spmd(
            _NC_CACHE, in_maps, core_ids=list(range(NCORES)), trace=trace)
        results = LAST_RUN.results
    return _assemble(results, np.asarray(b))


def _emulate(shards, wstat):
    """Host-side emulation of the device program (float64 accumulate)."""
    results = []
    for (u0, u1) in shards:
        full = np.concatenate([u0, u1[:, 2:]], axis=1)  # [128,18,18,NTT,NPH,SC]
        out = np.zeros((128, Y, Z, NTT, 2, SC), np.float64)
        for y in range(Y):
            ms = []
            for ph in range(NPH):
                acc = np.zeros((128, FREE), np.float64)
                for aidx, (ay, az) in enumerate(OFF9):
                    slab = full[:, y + ay, az:az + Z, :, ph, :].reshape(128, -1)
                    acc += wstat[:, ph * 9 + aidx, :].T.astype(np.float64) @ slab.astype(np.float64)
                ms.append(acc.reshape(128, Z, NTT, SC))
            out[:, y, :, :, 0, :] = ms[0] + ms[1] + ms[2]
            out[:, y, :, :, 1, :] = ms[1] - ms[2] - ms[3]
        results.append({"out": out.reshape(128, Y, Z, TLOC, SC)})
    return results


# revision 19
# speedup vs baseline: 1.6807x; 1.2127x over previous
"""4D circular cross-correlation (qcd_ml C_Convolution, k=3, nd=4) on 8 TRN2 cores.

Math: out[o, x,y,z,t, s,c] = b[o] + sum_{i, ax,ay,az,at} W[i,o,ax,ay,az,at]
                                   * U[i, x+ax-1, y+ay-1, z+az-1, t+at-1, s,c]
(all site indices circular). U complex64 (4,16,16,16,32,4,3), W complex64
(4,4,3,3,3,3), b complex64 (4,).

Device mapping (per core, T sharded 8-way with +-1 halos prepared on host):
  - contraction (matmul partition) dim = (reim_in 2, C_in 4, X 16) = 128
  - output (PSUM partition) dim       = (reim_out 2, C_out 4, X0 16) = 128
  - X offsets (ax) live inside the stationary 128x128 matrices, circularly
    banded in (x, x0); complex arithmetic is the 2x2 [[Wr, Wi], [-Wi, Wr]]
    block over the reim axes.
  - The T offsets (at) are removed by a host-side Winograd F(4,3) transform
    along t: the 4 local t outputs form ONE tile whose 6-point input window
    is exactly the t-halo slab; U becomes 6 phases (B^T d), weights become
    G W (6 phases); the device accumulates 9 (ay,az) offsets per phase into
    PSUM and combines the 6 phase results with A^T on the vector engine.
  - moving free dim = (y-pair 2, z 16, spin*color 12) = 384 <= 512 (one
    PSUM bank per phase).
  - y,z circular handled by host padding to 18; t halo from neighbor T-slab.
"""

import os
import sys
import itertools
import numpy as np

for _p in ("/opt/trn_rl_repo",):
    if _p not in sys.path and os.path.isdir(_p):
        sys.path.insert(0, _p)

C_IN, C_OUT = 4, 4
X = Y = Z = 16
T = 32
SC = 12  # spin*color
NCORES = 8
TLOC = T // NCORES          # 4 = one F(4,3) output tile
NPH = 6                     # Winograd F(4,3) phases
YPAD, ZPAD = Y + 2, Z + 2   # 18
UH_ROWS = 10                # y_pad rows per half tile (0..9 / 8..17)
OFF9 = list(itertools.product(range(3), repeat=2))  # (ay, az)
FREE = 2 * Z * SC           # 384, one chunk = (y-pair, z, sc) per phase

USE_FP32R = os.environ.get("CONV_FP32R", "1") == "1"

# Winograd F(4,3), points [0,1,-1,2,-2,inf] (correlation form:
# out[r] = sum_k g[k] d[r+k], r=0..3, d = U[t0-1 .. t0+4]).
BT = np.array([
    [4, 0, -5, 0, 1, 0],
    [0, -4, -4, 1, 1, 0],
    [0, 4, -4, -1, 1, 0],
    [0, -2, -1, 2, 1, 0],
    [0, 2, -1, -2, 1, 0],
    [0, 4, 0, -5, 0, 1]], np.float64)
G = np.array([
    [1 / 4, 0, 0],
    [-1 / 6, -1 / 6, -1 / 6],
    [-1 / 6, 1 / 6, -1 / 6],
    [1 / 24, 1 / 12, 1 / 6],
    [1 / 24, -1 / 12, 1 / 6],
    [0, 0, 1]], np.float64)
# A^T = [[1,1,1,1,1,0],
#        [0,1,-1,2,-2,0],
#        [0,1,1,4,4,0],
#        [0,1,-1,8,-8,1]]  -- applied on the device (DVE).


def _prep_u_shards(U):
    """U complex (4,16,16,16,32,4,3) -> per-core float32 y-halves
    [128, UH_ROWS, ZPAD, NPH, SC] of the t-Winograd-transformed field."""
    Ur = np.stack([U.real, U.imag], axis=0).astype(np.float32)  # (2,4,X,Y,Z,T,4,3)
    Ur = Ur.reshape(2, C_IN, X, Y, Z, T, SC)
    Up = np.pad(Ur, ((0, 0), (0, 0), (0, 0), (1, 1), (1, 1), (0, 0), (0, 0)),
                mode="wrap")  # (2,4,16,18,18,32,12)
    shards = []
    for k in range(NCORES):
        t0 = k * TLOC
        tidx = np.arange(t0 - 1, t0 + 5) % T        # 6-point window
        d = np.take(Up, tidx, axis=5)               # (2,4,16,18,18,6,12)
        m = np.einsum("pk,rixyzks->rixyzps", BT,
                      d.astype(np.float64)).astype(np.float32)
        m = m.reshape(128, YPAD, ZPAD, NPH, SC)
        u0 = np.ascontiguousarray(m[:, 0:UH_ROWS])
        u1 = np.ascontiguousarray(m[:, YPAD - UH_ROWS:YPAD])
        shards.append((u0, u1))
    return shards


def _prep_wstat(W):
    """W complex (4,4,3,3,3,3) -> [128, NPH*9, 128] float32 stationary stack.

    For phase p and (ay,az): Wg[p][i,o,ax,ay,az] = sum_at G[p,at] W[..,at];
    band in (x,x0): ax = (x - x0 + 1) mod 16 in {0,1,2};
    ri block M = [[Wr, Wi], [-Wi, Wr]] (columns riO: out_r, out_i).
    """
    Wc = np.ascontiguousarray(W).astype(np.complex128)
    Wg = np.einsum("pk,ioxyzk->pioxyz", G.astype(np.complex128), Wc)
    Wg = Wg.astype(np.complex64)                    # (6,4,4,3,3,3)
    stat = np.zeros((2, C_IN, X, NPH * 9, 2, C_OUT, X), np.float32)
    for ph in range(NPH):
        for aidx, (ay, az) in enumerate(OFF9):
            widx = ph * 9 + aidx
            for ax in range(3):
                wr = Wg[ph, :, :, ax, ay, az].real
                wi = Wg[ph, :, :, ax, ay, az].imag
                for x0 in range(X):
                    x = (x0 + ax - 1) % X
                    stat[0, :, x, widx, 0, :, x0] = wr
                    stat[1, :, x, widx, 0, :, x0] = -wi
                    stat[0, :, x, widx, 1, :, x0] = wi
                    stat[1, :, x, widx, 1, :, x0] = wr
    return np.ascontiguousarray(stat.reshape(128, NPH * 9, 128))


def _assemble(results, b):
    """results[k]["out"]: [128, Y, Z, TLOC, SC] f32 -> complex (4,16,16,16,32,4,3)."""
    out = np.empty((C_OUT, X, Y, Z, T, SC), np.complex64)
    for k in range(NCORES):
        r = np.asarray(results[k]["out"], np.float32).reshape(2, C_OUT, X, Y, Z, TLOC, SC)
        out[:, :, :, :, k * TLOC:(k + 1) * TLOC, :] = r[0] + 1j * r[1]
    out += np.asarray(b, np.complex64).reshape(C_OUT, 1, 1, 1, 1, 1)
    return np.ascontiguousarray(out.reshape(C_OUT, X, Y, Z, T, 4, 3))


def _build_nc():
    import concourse.mybir as mybir
    from concourse import bacc, tile
    from contextlib import ExitStack

    f32 = mybir.dt.float32
    mm_dt = mybir.dt.float32r if USE_FP32R else f32
    AluOp = mybir.AluOpType

    WCOLS = NPH * 9 * 128              # 6912
    UCOLS = UH_ROWS * ZPAD * NPH * SC  # 12960

    nc = bacc.Bacc()
    # wstat+u_half0 fused in one DMA (compute on y 0..7 starts as soon as it
    # lands); u_half1 is a separate DMA that overlaps with first-half compute
    # (HWDGE rings are FIFO, so the later-triggered u1 doesn't slow wu0).
    a_dram = nc.declare_dram_parameter("wu0", [128, WCOLS + UCOLS], mm_dt, isOutput=False)
    u1_dram = nc.declare_dram_parameter(
        "u1", [128, UH_ROWS, ZPAD, NPH, SC], mm_dt, isOutput=False)
    o_dram = nc.declare_dram_parameter("out", [128, Y, Z, TLOC, SC], f32, isOutput=True)

    with tile.TileContext(nc) as tc, ExitStack() as ctx:
        ipool = ctx.enter_context(tc.tile_pool(name="inp", bufs=1))
        opool = ctx.enter_context(tc.tile_pool(name="o", bufs=2))
        tpool = ctx.enter_context(tc.tile_pool(name="tmp", bufs=2))
        ppool = ctx.enter_context(tc.tile_pool(name="psum", bufs=8, space="PSUM"))

        big = ipool.tile([128, WCOLS + UCOLS], mm_dt, tag="wu0")
        nc.sync.dma_start(big[:], a_dram[:])
        u1t = ipool.tile([128, UH_ROWS, ZPAD, NPH, SC], mm_dt, tag="u1")
        nc.sync.dma_start(u1t[:], u1_dram[:])

        wt = big[:, :WCOLS].rearrange("p (a m) -> p a m", a=NPH * 9)
        ut = [
            big[:, WCOLS:].rearrange(
                "p (y z ph s) -> p y z ph s", y=UH_ROWS, z=ZPAD, ph=NPH),
            u1t,
        ]

        def stt(out_ap, sb_in, scalar, ps_or_sb):
            # out = (sb_in * scalar) +/- second operand, via scalar_tensor_tensor
            nc.vector.scalar_tensor_tensor(
                out_ap, in0=sb_in, scalar=scalar, in1=ps_or_sb,
                op0=AluOp.mult, op1=AluOp.add)

        YG = 4
        for g in range(Y // YG):
            ot = opool.tile([128, YG, Z, TLOC, SC], f32)
            for pair in range(YG // 2):
                y = g * YG + pair * 2               # even; pair (y, y+1)
                h = 0 if y < 8 else 1
                yl = y - 8 * h
                pts = []
                for ph in range(NPH):
                    pt = ppool.tile([128, FREE], f32)
                    for aidx, (ay, az) in enumerate(OFF9):
                        rhs = ut[h][:, yl + ay: yl + ay + 2, az: az + Z, ph, :]
                        nc.tensor.matmul(
                            pt[:],
                            wt[:, ph * 9 + aidx, :],
                            rhs,
                            start=(aidx == 0),
                            stop=(aidx == 8),
                        )
                    pts.append(pt)
                # A^T combine; every DVE op reads at most one PSUM operand.
                # b=m1+m2, a=m1-m2, u=m3+m4, s=m3-m4
                # t0=m0+b+u; t1=a+2s; t2=b+4u; t3=a+8s+m5
                ov = ot[:, pair * 2: pair * 2 + 2]  # [128, 2, Z, TLOC, SC]
                m1c = tpool.tile([128, FREE], f32, tag="m1c")
                nc.vector.tensor_copy(m1c[:], pts[1][:])
                bt_ = tpool.tile([128, FREE], f32, tag="bt")
                nc.vector.tensor_add(bt_[:], m1c[:], pts[2][:])
                m3c = tpool.tile([128, FREE], f32, tag="m3c")
                nc.vector.tensor_copy(m3c[:], pts[3][:])
                ut_ = tpool.tile([128, FREE], f32, tag="ut")
                nc.vector.tensor_add(ut_[:], m3c[:], pts[4][:])
                a_ = tpool.tile([128, FREE], f32, tag="at")
                nc.vector.scalar_tensor_tensor(
                    a_[:], in0=m1c[:], scalar=2.0, in1=bt_[:],
                    op0=AluOp.mult, op1=AluOp.subtract)
                s_ = tpool.tile([128, FREE], f32, tag="st")
                nc.vector.scalar_tensor_tensor(
                    s_[:], in0=m3c[:], scalar=2.0, in1=ut_[:],
                    op0=AluOp.mult, op1=AluOp.subtract)
                t0a = tpool.tile([128, FREE], f32, tag="t0a")
                nc.vector.tensor_add(t0a[:], bt_[:], pts[0][:])
                t3a = tpool.tile([128, FREE], f32, tag="t3a")
                nc.vector.scalar_tensor_tensor(
                    t3a[:], in0=s_[:], scalar=8.0, in1=a_[:],
                    op0=AluOp.mult, op1=AluOp.add)
                # writes into ot: view dims (y2, z, sc) at fixed t=r
                def ow(r):
                    return ov[:, :, :, r, :]
                nc.vector.tensor_add(ow(0), t0a[:], ut_[:])
                stt(ow(1), s_[:], 2.0, a_[:])
                stt(ow(2), ut_[:], 4.0, bt_[:])
                nc.vector.tensor_add(ow(3), t3a[:], pts[5][:])
            nc.sync.dma_start(o_dram[:, g * YG:(g + 1) * YG], ot[:])

    # Bacc defers register allocation and sync-wait splitting to finalize();
    # run_bass_via_pjrt serializes the module as-is, so finalize here.
    nc.finalize()
    return nc


_NC_CACHE = None
LAST_RUN = None  # BassKernelResults of the most recent device run (for test.py)


def kernel(U, W, b):
    global _NC_CACHE, LAST_RUN
    shards = _prep_u_shards(np.asarray(U))
    wstat = _prep_wstat(np.asarray(W))

    if os.environ.get("CONV_EMULATE", "0") == "1":
        results = _emulate(shards, wstat)
    else:
        from concourse.bass_utils import run_bass_kernel_spmd
        if _NC_CACHE is None:
            _NC_CACHE = _build_nc()
        wflat = wstat.reshape(128, -1)
        in_maps = [
            {"wu0": np.ascontiguousarray(np.concatenate(
                [wflat, u0.reshape(128, -1)], axis=1)),
             "u1": u1}
            for (u0, u1) in shards
        ]
        trace = os.environ.get("CONV_TRACE", "0") == "1"
        LAST_RUN = run_bass_kernel_spmd(
            _NC_CACHE, in_maps, core_ids=list(range(NCORES)), trace=trace)
        results = LAST_RUN.results
    return _assemble(results, np.asarray(b))


def _emulate(shards, wstat):
    """Host-side emulation of the device program (float64 accumulate)."""
    AT = np.array([
        [1, 1, 1, 1, 1, 0],
        [0, 1, -1, 2, -2, 0],
        [0, 1, 1, 4, 4, 0],
        [0, 1, -1, 8, -8, 1]], np.float64)
    results = []
    for (u0, u1) in shards:
        full = np.concatenate([u0, u1[:, 2:]], axis=1)  # [128,18,18,NPH,SC]
        out = np.zeros((128, Y, Z, TLOC, SC), np.float64)
        for y in range(0, Y, 2):
            ms = []
            for ph in range(NPH):
                acc = np.zeros((128, FREE), np.float64)
                for aidx, (ay, az) in enumerate(OFF9):
                    slab = full[:, y + ay: y + ay + 2, az:az + Z, ph, :].reshape(128, -1)
                    acc += wstat[:, ph * 9 + aidx, :].T.astype(np.float64) @ slab.astype(np.float64)
                ms.append(acc.reshape(128, 2, Z, SC))
            m = np.stack(ms, axis=0)  # (6, 128, 2, Z, SC)
            res = np.einsum("rp,pnyzs->nyzrs", AT, m)  # (128, 2, Z, 4, SC)
            out[:, y:y + 2, :, :, :] = res
        results.append({"out": out.reshape(128, Y, Z, TLOC, SC)})
    return results


# revision 24
# speedup vs baseline: 1.7063x; 1.0152x over previous
"""4D circular cross-correlation (qcd_ml C_Convolution, k=3, nd=4) on 8 TRN2 cores.

Math: out[o, x,y,z,t, s,c] = b[o] + sum_{i, ax,ay,az,at} W[i,o,ax,ay,az,at]
                                   * U[i, x+ax-1, y+ay-1, z+az-1, t+at-1, s,c]
(all site indices circular). U complex64 (4,16,16,16,32,4,3), W complex64
(4,4,3,3,3,3), b complex64 (4,).

Device mapping (per core, T sharded 8-way with +-1 halos prepared on host):
  - contraction (matmul partition) dim = (reim_in 2, C_in 4, X 16) = 128
  - output (PSUM partition) dim       = (reim_out 2, C_out 4, X0 16) = 128
  - X offsets (ax) live inside the stationary 128x128 matrices, circularly
    banded in (x, x0); complex arithmetic is the 2x2 [[Wr, Wi], [-Wi, Wr]]
    block over the reim axes.
  - The T offsets (at) are removed by a host-side Winograd F(4,3) transform
    along t: the 4 local t outputs form ONE tile whose 6-point input window
    is exactly the t-halo slab; U becomes 6 phases (B^T d), weights become
    G W (6 phases); the device accumulates 9 (ay,az) offsets per phase into
    PSUM and combines the 6 phase results with A^T on the vector engine.
  - moving free dim = (y-pair 2, z 16, spin*color 12) = 384 <= 512 (one
    PSUM bank per phase).
  - y,z circular handled by host padding to 18; t halo from neighbor T-slab.
"""

import os
import sys
import itertools
import numpy as np

for _p in ("/opt/trn_rl_repo",):
    if _p not in sys.path and os.path.isdir(_p):
        sys.path.insert(0, _p)

C_IN, C_OUT = 4, 4
X = Y = Z = 16
T = 32
SC = 12  # spin*color
NCORES = 8
TLOC = T // NCORES          # 4 = one F(4,3) output tile
NPH = 6                     # Winograd F(4,3) phases
YPAD, ZPAD = Y + 2, Z + 2   # 18
UH_ROWS = 10                # y_pad rows per half tile (0..9 / 8..17)
OFF9 = list(itertools.product(range(3), repeat=2))  # (ay, az)
FREE = 2 * Z * SC           # 384, one chunk = (y-pair, z, sc) per phase

USE_FP32R = os.environ.get("CONV_FP32R", "1") == "1"

# Winograd F(4,3), points [0,1,-1,2,-2,inf] (correlation form:
# out[r] = sum_k g[k] d[r+k], r=0..3, d = U[t0-1 .. t0+4]).
BT = np.array([
    [4, 0, -5, 0, 1, 0],
    [0, -4, -4, 1, 1, 0],
    [0, 4, -4, -1, 1, 0],
    [0, -2, -1, 2, 1, 0],
    [0, 2, -1, -2, 1, 0],
    [0, 4, 0, -5, 0, 1]], np.float64)
G = np.array([
    [1 / 4, 0, 0],
    [-1 / 6, -1 / 6, -1 / 6],
    [-1 / 6, 1 / 6, -1 / 6],
    [1 / 24, 1 / 12, 1 / 6],
    [1 / 24, -1 / 12, 1 / 6],
    [0, 0, 1]], np.float64)
# A^T = [[1,1,1,1,1,0],
#        [0,1,-1,2,-2,0],
#        [0,1,1,4,4,0],
#        [0,1,-1,8,-8,1]]  -- applied on the device (DVE).


def _prep_u_shards(U):
    """U complex (4,16,16,16,32,4,3) -> per-core float32 y-quarters
    [128, 6, ZPAD, NPH, SC] of the t-Winograd-transformed field
    (quarter q = y_pad rows 4q..4q+6, serving output y 4q..4q+4)."""
    Ur = np.stack([U.real, U.imag], axis=0).astype(np.float32)  # (2,4,X,Y,Z,T,4,3)
    Ur = Ur.reshape(2, C_IN, X, Y, Z, T, SC)
    Up = np.pad(Ur, ((0, 0), (0, 0), (0, 0), (1, 1), (1, 1), (0, 0), (0, 0)),
                mode="wrap")  # (2,4,16,18,18,32,12)
    shards = []
    for k in range(NCORES):
        t0 = k * TLOC
        tidx = np.arange(t0 - 1, t0 + 5) % T        # 6-point window
        d = np.take(Up, tidx, axis=5)               # (2,4,16,18,18,6,12)
        m = np.einsum("pk,rixyzks->rixyzps", BT,
                      d.astype(np.float64)).astype(np.float32)
        m = m.reshape(128, YPAD, ZPAD, NPH, SC)
        shards.append(tuple(
            np.ascontiguousarray(m[:, 4 * q: 4 * q + 6]) for q in range(4)))
    return shards


def _prep_wstat(W):
    """W complex (4,4,3,3,3,3) -> [128, NPH*9, 128] float32 stationary stack.

    For phase p and (ay,az): Wg[p][i,o,ax,ay,az] = sum_at G[p,at] W[..,at];
    band in (x,x0): ax = (x - x0 + 1) mod 16 in {0,1,2};
    ri block M = [[Wr, Wi], [-Wi, Wr]] (columns riO: out_r, out_i).
    """
    Wc = np.ascontiguousarray(W).astype(np.complex128)
    Wg = np.einsum("pk,ioxyzk->pioxyz", G.astype(np.complex128), Wc)
    Wg = Wg.astype(np.complex64)                    # (6,4,4,3,3,3)
    stat = np.zeros((2, C_IN, X, NPH * 9, 2, C_OUT, X), np.float32)
    for ph in range(NPH):
        for aidx, (ay, az) in enumerate(OFF9):
            widx = ph * 9 + aidx
            for ax in range(3):
                wr = Wg[ph, :, :, ax, ay, az].real
                wi = Wg[ph, :, :, ax, ay, az].imag
                for x0 in range(X):
                    x = (x0 + ax - 1) % X
                    stat[0, :, x, widx, 0, :, x0] = wr
                    stat[1, :, x, widx, 0, :, x0] = -wi
                    stat[0, :, x, widx, 1, :, x0] = wi
                    stat[1, :, x, widx, 1, :, x0] = wr
    return np.ascontiguousarray(stat.reshape(128, NPH * 9, 128))


def _assemble(results, b):
    """results[k]["out"]: [128, Y, Z, TLOC, SC] f32 -> complex (4,16,16,16,32,4,3)."""
    out = np.empty((C_OUT, X, Y, Z, T, SC), np.complex64)
    for k in range(NCORES):
        r = np.asarray(results[k]["out"], np.float32).reshape(2, C_OUT, X, Y, Z, TLOC, SC)
        out[:, :, :, :, k * TLOC:(k + 1) * TLOC, :] = r[0] + 1j * r[1]
    out += np.asarray(b, np.complex64).reshape(C_OUT, 1, 1, 1, 1, 1)
    return np.ascontiguousarray(out.reshape(C_OUT, X, Y, Z, T, 4, 3))


def _build_nc():
    import concourse.mybir as mybir
    from concourse import bacc, tile
    from contextlib import ExitStack

    f32 = mybir.dt.float32
    mm_dt = mybir.dt.float32r if USE_FP32R else f32
    AluOp = mybir.AluOpType

    WCOLS = NPH * 9 * 128              # 6912
    UCOLS = UH_ROWS * ZPAD * NPH * SC  # 12960

    QROWS = 6  # y_pad rows per quarter tile: quarter q holds rows 4q..4q+6

    nc = bacc.Bacc()
    # Input split: wstat first, then four y-quarter slabs of the transformed
    # field, so matmuls on y 0..3 start after ~2 small DMAs; later quarters
    # stream in under compute (HWDGE rings are FIFO, so later-triggered DMAs
    # don't slow earlier ones).
    w_dram = nc.declare_dram_parameter("wstat", [128, NPH * 9, 128], mm_dt, isOutput=False)
    q_dram = [
        nc.declare_dram_parameter(f"uq{q}", [128, QROWS, ZPAD, NPH, SC], mm_dt, isOutput=False)
        for q in range(4)
    ]
    o_dram = nc.declare_dram_parameter("out", [128, Y, Z, TLOC, SC], f32, isOutput=True)

    with tile.TileContext(nc) as tc, ExitStack() as ctx:
        ipool = ctx.enter_context(tc.tile_pool(name="inp", bufs=1))
        opool = ctx.enter_context(tc.tile_pool(name="o", bufs=2))
        tpool = ctx.enter_context(tc.tile_pool(name="tmp", bufs=1))
        ppool = ctx.enter_context(tc.tile_pool(name="psum", bufs=8, space="PSUM"))

        wt = ipool.tile([128, NPH * 9, 128], mm_dt, tag="w")
        nc.sync.dma_start(wt[:], w_dram[:])
        uq = []
        for q in range(4):
            t = ipool.tile([128, QROWS, ZPAD, NPH, SC], mm_dt, tag=f"uq{q}")
            nc.sync.dma_start(t[:], q_dram[q][:])
            uq.append(t)

        def stt(out_ap, sb_in, scalar, ps_or_sb):
            # out = (sb_in * scalar) +/- second operand, via scalar_tensor_tensor
            nc.vector.scalar_tensor_tensor(
                out_ap, in0=sb_in, scalar=scalar, in1=ps_or_sb,
                op0=AluOp.mult, op1=AluOp.add)

        YG = 4
        for g in range(Y // YG):
            ot = opool.tile([128, YG, Z, TLOC, SC], f32)
            for pair in range(YG // 2):
                y = g * YG + pair * 2               # even; pair (y, y+1)
                q = y // 4
                yl = y - 4 * q
                pts = []
                for ph in range(NPH):
                    pt = ppool.tile([128, FREE], f32)
                    for aidx, (ay, az) in enumerate(OFF9):
                        rhs = uq[q][:, yl + ay: yl + ay + 2, az: az + Z, ph, :]
                        nc.tensor.matmul(
                            pt[:],
                            wt[:, ph * 9 + aidx, :],
                            rhs,
                            start=(aidx == 0),
                            stop=(aidx == 8),
                        )
                    pts.append(pt)
                # A^T combine; every DVE op reads at most one PSUM operand.
                # b=m1+m2, a=m1-m2, u=m3+m4, s=m3-m4
                # t0=m0+b+u; t1=a+2s; t2=b+4u; t3=a+8s+m5
                # Ordered so PSUM banks m1,m2,m0,m3,m4 free as early as
                # possible (the next pair's matmuls reuse them).
                ov = ot[:, pair * 2: pair * 2 + 2]  # [128, 2, Z, TLOC, SC]
                m1c = tpool.tile([128, FREE], f32, tag="m1c")
                nc.vector.tensor_copy(m1c[:], pts[1][:])
                bt_ = tpool.tile([128, FREE], f32, tag="bt")
                nc.vector.tensor_add(bt_[:], m1c[:], pts[2][:])
                t0a = tpool.tile([128, FREE], f32, tag="t0a")
                nc.vector.tensor_add(t0a[:], bt_[:], pts[0][:])
                m3c = tpool.tile([128, FREE], f32, tag="m3c")
                nc.vector.tensor_copy(m3c[:], pts[3][:])
                ut_ = tpool.tile([128, FREE], f32, tag="ut")
                nc.vector.tensor_add(ut_[:], m3c[:], pts[4][:])
                a_ = tpool.tile([128, FREE], f32, tag="at")
                nc.vector.scalar_tensor_tensor(
                    a_[:], in0=m1c[:], scalar=2.0, in1=bt_[:],
                    op0=AluOp.mult, op1=AluOp.subtract)
                s_ = tpool.tile([128, FREE], f32, tag="st")
                nc.vector.scalar_tensor_tensor(
                    s_[:], in0=m3c[:], scalar=2.0, in1=ut_[:],
                    op0=AluOp.mult, op1=AluOp.subtract)
                t3a = tpool.tile([128, FREE], f32, tag="t3a")
                nc.vector.scalar_tensor_tensor(
                    t3a[:], in0=s_[:], scalar=8.0, in1=a_[:],
                    op0=AluOp.mult, op1=AluOp.add)
                # writes into ot: view dims (y2, z, sc) at fixed t=r
                def ow(r):
                    return ov[:, :, :, r, :]
                nc.vector.tensor_add(ow(0), t0a[:], ut_[:])
                stt(ow(1), s_[:], 2.0, a_[:])
                stt(ow(2), ut_[:], 4.0, bt_[:])
                nc.vector.tensor_add(ow(3), t3a[:], pts[5][:])
            nc.sync.dma_start(o_dram[:, g * YG:(g + 1) * YG], ot[:])

    # Bacc defers register allocation and sync-wait splitting to finalize();
    # run_bass_via_pjrt serializes the module as-is, so finalize here.
    nc.finalize()
    return nc


_NC_CACHE = None
LAST_RUN = None  # BassKernelResults of the most recent device run (for test.py)


def kernel(U, W, b):
    global _NC_CACHE, LAST_RUN
    shards = _prep_u_shards(np.asarray(U))
    wstat = _prep_wstat(np.asarray(W))

    if os.environ.get("CONV_EMULATE", "0") == "1":
        results = _emulate(shards, wstat)
    else:
        from concourse.bass_utils import run_bass_kernel_spmd
        if _NC_CACHE is None:
            _NC_CACHE = _build_nc()
        in_maps = [
            {"wstat": wstat, **{f"uq{q}": qs[q] for q in range(4)}}
            for qs in shards
        ]
        trace = os.environ.get("CONV_TRACE", "0") == "1"
        LAST_RUN = run_bass_kernel_spmd(
            _NC_CACHE, in_maps, core_ids=list(range(NCORES)), trace=trace)
        results = LAST_RUN.results
    return _assemble(results, np.asarray(b))


def _emulate(shards, wstat):
    """Host-side emulation of the device program (float64 accumulate)."""
    AT = np.array([
        [1, 1, 1, 1, 1, 0],
        [0, 1, -1, 2, -2, 0],
        [0, 1, 1, 4, 4, 0],
        [0, 1, -1, 8, -8, 1]], np.float64)
    results = []
    for qs in shards:
        out = np.zeros((128, Y, Z, TLOC, SC), np.float64)
        for y in range(0, Y, 2):
            q, yl = y // 4, y % 4
            ms = []
            for ph in range(NPH):
                acc = np.zeros((128, FREE), np.float64)
                for aidx, (ay, az) in enumerate(OFF9):
                    slab = qs[q][:, yl + ay: yl + ay + 2, az:az + Z, ph, :].reshape(128, -1)
                    acc += wstat[:, ph * 9 + aidx, :].T.astype(np.float64) @ slab.astype(np.float64)
                ms.append(acc.reshape(128, 2, Z, SC))
            m = np.stack(ms, axis=0)  # (6, 128, 2, Z, SC)
            res = np.einsum("rp,pnyzs->nyzrs", AT, m)  # (128, 2, Z, 4, SC)
            out[:, y:y + 2, :, :, :] = res
        results.append({"out": out.reshape(128, Y, Z, TLOC, SC)})
    return results
